# revision 5
# baseline (speedup 1.0000x reference)
"""Trainium2 Bass kernel for 3x3 same-padding Conv2d on [4, 4096, 4096] fp32.

Strategy:
  - Shard H across 8 NeuronCores (512 output rows each) with 1-row halos,
    host-side. W is padded by 1 on each side host-side too, so the device
    program needs no edge special-casing.
  - On each core, the conv is computed on the TensorEngine as banded-Toeplitz
    matmuls: for an output block of 126 rows, the stationary operand is a
    [K=128(input rows), M=126(output rows)] band matrix holding the 3 dy-taps
    of weight k[co, ci, :, dx]; the moving operand is the input tile
    [128 rows, 512 w-positions] shifted by dx in the free dim. Accumulating
    over (ci, dx) = 12 matmuls yields one [126, 512] output chunk in PSUM.
  - The 8-row tail block (512 = 4*126 + 8) packs all 4 ci into the partition
    dim (K = 4ci x 10 rows = 40) so it needs only 3 matmuls (dx) per chunk.
  - PSUM -> SBUF via VectorEngine copies, then HWDGE DMA back to HBM.

Band matrices are built host-side from the conv weight and passed as inputs.
"""

import numpy as np

import concourse.bass as bass
import concourse.tile as tile
from concourse import bacc, mybir
from concourse.bass_utils import run_bass_kernel_spmd

N_CORES = 8
C = 4                    # channels (in = out = 4)
H = 4096
W = 4096
SH = H // N_CORES        # 512 output rows per core
YB = 126                 # full-block output rows
N_FULL = SH // YB        # 4 full blocks
TAIL = SH - N_FULL * YB  # 8 tail rows
WH = 2048                # W half processed per X-tile residency
WC = 512                 # matmul free size / PSUM bank width

# dtype used for the matmul operands (stationary + moving).
# float32r = 4-byte fp32 layout, runs at 1 cycle/row on the PE (vs 4 for f32).
MM_DT = mybir.dt.float32r
OUT_DT = mybir.dt.float32

_CACHE = {}


def _build_program():
    nc = bacc.Bacc(
        "TRN2", target_bir_lowering=False, debug=False, num_devices=N_CORES
    )

    xs_d = nc.dram_tensor("xs", [C, SH + 2, W + 2], MM_DT, kind="ExternalInput")
    bands_d = nc.dram_tensor("bands", [128, 48 * YB], MM_DT, kind="ExternalInput")
    tails_d = nc.dram_tensor(
        "tails", [C * (TAIL + 2), 12 * TAIL], MM_DT, kind="ExternalInput"
    )
    ys_d = nc.dram_tensor("ys", [C, SH, W], OUT_DT, kind="ExternalOutput")

    xs = xs_d.ap()
    ys = ys_d.ap()

    with tile.TileContext(nc) as tc:
        with (
            tc.tile_pool(name="bp", bufs=1) as bpool,
            tc.tile_pool(name="xp", bufs=8) as xpool,
            tc.tile_pool(name="op", bufs=8) as opool,
            tc.tile_pool(name="pp", bufs=8, space=bass.MemorySpace.PSUM) as ppool,
        ):
            bt = bpool.tile([128, 48 * YB], MM_DT, tag="bands")
            nc.sync.dma_start(out=bt[:], in_=bands_d.ap()[:])
            tt = bpool.tile([C * (TAIL + 2), 12 * TAIL], MM_DT, tag="tails")
            nc.sync.dma_start(out=tt[:], in_=tails_d.ap()[:])

            for yb in range(N_FULL):
                r0 = YB * yb
                for wh in range(2):
                    c0 = WH * wh
                    X = []
                    for ci in range(C):
                        xt = xpool.tile([128, WH + 2], MM_DT, tag="xt")
                        nc.sync.dma_start(
                            out=xt[:],
                            in_=xs[ci, r0 : r0 + 128, c0 : c0 + WH + 2],
                        )
                        X.append(xt)
                    for co in range(C):
                        ot = opool.tile([YB, WH], OUT_DT, tag="ot")
                        pss = [
                            ppool.tile([YB, WC], mybir.dt.float32, tag="ps", name=f"ps{i}")
                            for i in range(WH // WC)
                        ]
                        # (ci, dx) outer so the stationary band is loaded once
                        # per 4 matmuls (one per wc chunk)
                        for idx in range(12):
                            ci, dx = divmod(idx, 3)
                            b = (co * C + ci) * 3 + dx
                            for wc in range(WH // WC):
                                nc.tensor.matmul(
                                    pss[wc][:],
                                    bt[:, b * YB : (b + 1) * YB],
                                    X[ci][:, WC * wc + dx : WC * wc + dx + WC],
                                    start=(idx == 0),
                                    stop=(idx == 11),
                                )
                        for wc in range(WH // WC):
                            nc.vector.tensor_copy(
                                ot[:, WC * wc : WC * (wc + 1)], pss[wc][:]
                            )
                        nc.sync.dma_start(
                            out=ys[co, r0 : r0 + YB, c0 : c0 + WH], in_=ot[:]
                        )

            # tail block: output rows [504, 512), K packs (ci, 10 input rows)
            r0 = YB * N_FULL
            K_T = TAIL + 2
            for wh in range(2):
                c0 = WH * wh
                xtt = xpool.tile([C * K_T, WH + 2], MM_DT, tag="xt")
                for ci in range(C):
                    nc.sync.dma_start(
                        out=xtt[K_T * ci : K_T * (ci + 1), :],
                        in_=xs[ci, r0 : r0 + K_T, c0 : c0 + WH + 2],
                    )
                for co in range(C):
                    ot = opool.tile([TAIL, WH], OUT_DT, tag="ot")
                    pss = [
                        ppool.tile([TAIL, WC], mybir.dt.float32, tag="ps", name=f"pst{i}")
                        for i in range(WH // WC)
                    ]
                    for dx in range(3):
                        b = co * 3 + dx
                        for wc in range(WH // WC):
                            nc.tensor.matmul(
                                pss[wc][:],
                                tt[:, b * TAIL : (b + 1) * TAIL],
                                xtt[:, WC * wc + dx : WC * wc + dx + WC],
                                start=(dx == 0),
                                stop=(dx == 2),
                            )
                    for wc in range(WH // WC):
                        nc.vector.tensor_copy(ot[:, WC * wc : WC * (wc + 1)], pss[wc][:])
                    nc.sync.dma_start(
                        out=ys[co, r0 : r0 + TAIL, c0 : c0 + WH], in_=ot[:]
                    )

    nc.compile()
    return nc


def _make_bands(kw: np.ndarray) -> tuple[np.ndarray, np.ndarray]:
    """kw: [co, ci, 3, 3] -> (bands [128, 48*126], tails [40, 12*8])."""
    bands = np.zeros((128, 48 * YB), dtype=np.float32)
    for co in range(C):
        for ci in range(C):
            for dx in range(3):
                b = (co * C + ci) * 3 + dx
                blk = np.zeros((128, YB), dtype=np.float32)
                for dy in range(3):
                    # column j' (output row) gets weight at partition j' + dy
                    idx = np.arange(YB)
                    blk[idx + dy, idx] = kw[co, ci, dy, dx]
                bands[:, b * YB : (b + 1) * YB] = blk
    K_T = TAIL + 2
    tails = np.zeros((C * K_T, 12 * TAIL), dtype=np.float32)
    for co in range(C):
        for dx in range(3):
            b = co * 3 + dx
            blk = np.zeros((C * K_T, TAIL), dtype=np.float32)
            for ci in range(C):
                for dy in range(3):
                    idx = np.arange(TAIL)
                    blk[K_T * ci + idx + dy, idx] = kw[co, ci, dy, dx]
            tails[:, b * TAIL : (b + 1) * TAIL] = blk
    return bands, tails


def kernel(x: np.ndarray, kernel: np.ndarray) -> np.ndarray:
    x = np.asarray(x, dtype=np.float32)
    kw = np.asarray(kernel, dtype=np.float32)

    if "nc" not in _CACHE:
        _CACHE["nc"] = _build_program()
    nc = _CACHE["nc"]

    xpad = np.zeros((C, H + 2, W + 2), dtype=np.float32)
    xpad[:, 1 : H + 1, 1 : W + 1] = x
    bands, tails = _make_bands(kw)

    in_maps = []
    for c in range(N_CORES):
        in_maps.append(
            {
                "xs": np.ascontiguousarray(xpad[:, SH * c : SH * c + SH + 2, :]),
                "bands": bands,
                "tails": tails,
            }
        )

    res = run_bass_kernel_spmd(nc, in_maps, list(range(N_CORES)))
    out = np.concatenate([res.results[c]["ys"] for c in range(N_CORES)], axis=1)
    return out
